# revision 1
# baseline (speedup 1.0000x reference)
"""HMM forward-algorithm (nn_ForwardBackward) Trainium2 Bass kernel.

The reference is a strictly sequential scan of T=8192 steps, each a matvec of
the state row-vector with the SxS transition matrix followed by a rescale.
Rewritten in linear space:

    s_t = (s_{t-1} @ A) * E_t / Z_{t-1},   Z_t = sum(s_t),  E_t = exp(probt[t])
    LL  = sum_t log Z_t                    (s_0 = clip(pi,eps) * E_0)

Tensor-parallel over 8 NeuronCores: core k owns a 256-column slice of A
(SBUF-resident, fp8-e4m3).  Per step each core computes its 256 outputs with
weights-stationary fp8 matmuls (A blocks [128,128] stationary, state chunks
[128,1] moving, partition-major [128,1] outputs accumulated over the
contraction) so the result lands directly in the layout the exchange needs —
no transpose.  DVE applies the E/Z factor; each core broadcasts its fp8
[128,2] slice (16B-strided into the stationary state layout) to all 8 cores
with SWDGE remote DMA (XOR-slot addressing, per-slot parity-split semaphores,
runtime probe for the logical->physical core permutation).  Z comes from
accumulating ones-matmuls; Z values are logged per step and the log-sum is
folded once in a bulk epilogue.  Everything (A slice, exp(probt) slice, state
buffers) lives in SBUF; no per-step HBM traffic.  fp8 end-to-end relative
error on the final LL is ~1e-4.
"""

import functools
import numpy as np
from ml_dtypes import float8_e4m3

from concourse import bass, bacc, mybir
from concourse.bass_utils import run_bass_kernel_spmd

S = 2048
T = 8192
NC = 8
SL = S // NC          # 256 state columns per core
CH = 16               # contraction chunks of 128
DC = CH // 2          # DoubleRow chunk pairs
G = 4                 # col-tile groups
NG = SL // G          # 64 output columns per group
F = 2                 # send-buffer free columns (256 = 128*2)
EPS = 1e-8

f32 = mybir.dt.float32
f8 = mybir.dt.float8e4


def _jlocal():
    """local element j at (partition p, send col c): j = 128*c + p."""
    p = np.arange(128)
    jl = np.zeros((128, F), dtype=np.int64)
    for c in range(F):
        jl[:, c] = 128 * c + p
    return jl


def _build_probe():
    """Tiny SPMD kernel: every core broadcasts its id-tile to all 8 XOR slots."""
    nc = bacc.Bacc(None, target_bir_lowering=False, num_devices=NC)
    x = nc.declare_dram_parameter("x", [128, 1], f32, isOutput=False)
    y = nc.declare_dram_parameter("y", [128, NC], f32, isOutput=True)
    with (
        nc.sbuf_tensor("S", [128, 1], f32) as Sb,
        nc.sbuf_tensor("R", [128, NC], f32) as Rb,
        nc.semaphore("dma_sem") as dma_sem,
        nc.semaphore("recv_sem") as recv_sem,
        nc.semaphore("send_local") as send_local,
        nc.semaphore("prep_sem") as prep_sem,
        nc.semaphore("out_sem") as out_sem,
        nc.Block() as block,
    ):
        @block.sync
        def _(sync):
            sync.dma_start(Sb[:, :], x[:, :]).then_inc(dma_sem, 16)

        @block.gpsimd
        def _(gp):
            for d in range(NC):
                rdests = [None] * NC
                rdests[d] = (0, d)
                gp.remote_dma_broadcast(
                    out_ap=Rb[:, d:d + 1], in_ap=Sb[:, :],
                    remote_sem=recv_sem, local_sem=send_local, rdests=rdests,
                ).then_inc(prep_sem, 1)
            gp.wait_ge(prep_sem, NC)
            gp.wait_ge(dma_sem, 16)
            gp.trigger_dma(count=NC)
            gp.wait_ge(recv_sem, 16)
            gp.dma_start(y[:, :], Rb[:, :]).then_inc(out_sem, 16)
            gp.wait_ge(out_sem, 16)
            gp.wait_ge(send_local, 16 * NC)
    nc.finalize()
    return nc


@functools.lru_cache(maxsize=1)
def _probe_sigma():
    """sigma[j][d] = logical id of the core whose slice lands in slot d of core j."""
    nc = _build_probe()
    ins = [{"x": np.full((128, 1), k, np.float32)} for k in range(NC)]
    res = run_bass_kernel_spmd(nc, ins, core_ids=list(range(NC)))
    sigma = []
    for j in range(NC):
        out = res.results[j]["y"]
        sigma.append(tuple(int(out[0, d]) for d in range(NC)))
    return tuple(sigma)


@functools.lru_cache(maxsize=8)
def _build_main(nsteps):
    """Full HMM forward kernel, unrolled over `nsteps` total steps (incl. t=0).

    Steps t = 1 .. nsteps-1 are compute+exchange rounds; step t == nsteps is an
    epilogue Z-only round; then a bulk log-sum epilogue produces the output.
    """
    Tm = nsteps
    TmPad = ((Tm + 1 + 127) // 128) * 128
    ZCOLS = TmPad // 128
    nc = bacc.Bacc(None, target_bir_lowering=False, num_devices=NC)

    a_d = nc.declare_dram_parameter("a", [128, CH * 2 * 128], f8, isOutput=False)
    e_d = nc.declare_dram_parameter("e", [128, F * Tm], f32, isOutput=False)
    r0_d = nc.declare_dram_parameter("r0", [128, 16 * CH], f8, isOutput=False)
    out_d = nc.declare_dram_parameter("out", [1, 1], f32, isOutput=True)

    from contextlib import ExitStack
    with ExitStack() as es:
        ec = es.enter_context
        Abuf = ec(nc.sbuf_tensor("Abuf", [128, CH * 2 * 128], f8))
        Ebuf = ec(nc.sbuf_tensor("Ebuf", [128, F * Tm], f32))
        R = [ec(nc.sbuf_tensor(f"R{i}", [128, 16 * CH], f8)) for i in range(2)]
        Sb = [ec(nc.sbuf_tensor(f"S{i}", [128, 32], f8)) for i in range(2)]
        Tb = ec(nc.sbuf_tensor("Tb", [128, NG], f32))
        invZ = ec(nc.sbuf_tensor("invZ", [128, 1], f32))
        Et = ec(nc.sbuf_tensor("Et", [128, F], f32))
        zlog = ec(nc.sbuf_tensor("zlog", [1, TmPad], f32))
        zlog2 = ec(nc.sbuf_tensor("zlog2", [128, ZCOLS], f32))
        onesw = ec(nc.sbuf_tensor("onesw", [128, 1], f32))
        LLacc = ec(nc.sbuf_tensor("LLacc", [1, 1], f32))
        ones = ec(nc.sbuf_tensor("ones", [128, 2 * 128], f8))  # plain fp8 ones
        P = [[ec(nc.psum_tensor(f"P{i}_{h}", [128, 512], f32)) for h in range(2)] for i in range(2)]
        Zp = [ec(nc.psum_tensor(f"Zp{i}", [128, 512], f32)) for i in range(2)]
        LLp = ec(nc.psum_tensor("LLp", [1, 512], f32))
        sem = lambda n: ec(nc.semaphore(n))
        ld_sem = sem("ld_sem"); exp_sem = sem("exp_sem"); misc_sem = sem("misc_sem")
        zmm_sem = sem("zmm_sem"); mm_sem = sem("mm_sem"); mmh_sem = sem("mmh_sem")
        dve_sem = sem("dve_sem"); prep_sem = sem("prep_sem"); sloc_sem = sem("sloc_sem")
        out_sem = sem("out_sem"); ready_sem = sem("ready_sem"); fin_sem = sem("fin_sem")
        # remote-arrival sems, split by round parity: round t updates
        # rv[t%2][d] from 2n-2 to 2n with n = (t+1)//2.
        rv = [[sem(f"rv{p}_{d}") for d in range(NC)] for p in range(2)]
        block = ec(nc.Block())
        EXP_CHUNK = 1024
        n_exp = (F * Tm + EXP_CHUNK - 1) // EXP_CHUNK

        @block.sync
        def _(sync):
            sync.dma_start(Abuf[:, :], a_d[:, :]).then_inc(ld_sem, 16)
            sync.dma_start(Ebuf[:, :], e_d[:, :]).then_inc(ld_sem, 16)
            sync.dma_start(R[1][:, :], r0_d[:, :]).then_inc(ld_sem, 16)

        @block.scalar
        def _(act):
            act.wait_ge(ld_sem, 48)
            for i in range(n_exp):
                lo = i * EXP_CHUNK
                hi = min(lo + EXP_CHUNK, F * Tm)
                act.activation(
                    Ebuf[:, lo:hi], Ebuf[:, lo:hi],
                    mybir.ActivationFunctionType.Exp,
                ).then_inc(exp_sem, 1)
            # bulk log-sum epilogue: ln(zlog2) in place
            act.wait_ge(fin_sem, 16)
            act.activation(
                zlog2[:, :], zlog2[:, :], mybir.ActivationFunctionType.Ln,
            ).then_inc(exp_sem, 1)

        @block.tensor
        def _(te):
            te.wait_ge(ld_sem, 48)
            te.wait_ge(misc_sem, 3)
            for t in range(1, Tm + 1):
                bt = t % 2
                if t >= 2:
                    te.wait_ge(ready_sem, t - 1)
                if t >= 3:
                    te.wait_ge(dve_sem, t - 2)   # Zp/P[bt] WAR vs DVE reads
                lastz = None
                for c in range(CH):
                    lastz = nc.tensor.matmul(
                        Zp[bt][:, 0:1], ones[:, 0:128],
                        R[bt][:, 16 * c:16 * c + 1],
                        start=(c == 0), stop=(c == CH - 1),
                    )
                lastz.then_inc(zmm_sem, 1)
                if t == Tm:
                    break
                lasth = [None, None]
                for c in range(CH):
                    for h in range(2):
                        lasth[h] = nc.tensor.matmul(
                            P[bt][h][:, 0:1],
                            Abuf[:, (c * 2 + h) * 128:(c * 2 + h + 1) * 128],
                            R[bt][:, 16 * c:16 * c + 1],
                            start=(c == 0), stop=(c == CH - 1),
                        )
                lasth[0].then_inc(mmh_sem, 1)
                lasth[1].then_inc(mm_sem, 1)
            # bulk epilogue: sum of ln(Z) via ones-matvec, after ACT's Ln
            te.wait_ge(exp_sem, n_exp + 1)
            nc.tensor.matmul(
                LLp[0:1, 0:ZCOLS], onesw[:, 0:1], zlog2[:, :], start=True, stop=True,
            ).then_inc(zmm_sem, 1)

        @block.vector
        def _(ve):
            ve.memset(ones[:, :], 1.0).then_inc(misc_sem, 1)
            ve.memset(P[0][0][:, :], 1.0)
            ve.memset(P[0][1][:, :], 1.0)
            ve.memset(P[1][0][:, :], 1.0).then_inc(misc_sem, 1)
            ve.memset(P[1][1][:, :], 1.0).then_inc(misc_sem, 1)
            ve.memset(Sb[0][:, :], 0.0)
            ve.memset(Sb[1][:, :], 0.0)
            ve.memset(zlog[:, :], 1.0)
            ve.memset(onesw[:, :], 1.0)
            ve.drain()
            ve.wait_ge(exp_sem, n_exp)
            for t in range(1, Tm + 1):
                bt = t % 2
                ve.wait_ge(zmm_sem, t)
                if t == Tm:
                    ve.tensor_copy(
                        zlog[0:1, t:t + 1], Zp[bt][0:1, 0:1],
                    ).then_inc(dve_sem, 1)
                    break
                ve.reciprocal(invZ[:, :], Zp[bt][:, 0:1])
                ve.drain()
                ve.tensor_scalar_mul(
                    Et[:, :], Ebuf[:, F * t:F * (t + 1)], invZ[:, 0:1])
                ve.tensor_copy(zlog[0:1, t:t + 1], Zp[bt][0:1, 0:1])
                ve.drain()
                if t >= 3:
                    ve.wait_ge(sloc_sem, 16 * NC * (t - 2))
                ve.wait_ge(mmh_sem, t)
                ve.tensor_tensor(
                    Sb[bt][:, 0:1], P[bt][0][:, 0:1], Et[:, 0:1],
                    op=mybir.AluOpType.mult,
                )
                ve.wait_ge(mm_sem, t)
                ve.tensor_tensor(
                    Sb[bt][:, 16:17], P[bt][1][:, 0:1], Et[:, 1:2],
                    op=mybir.AluOpType.mult,
                ).then_inc(dve_sem, 1)
            # epilogue: final LL = sum over LLp row
            ve.wait_ge(zmm_sem, Tm + 1)
            ve.tensor_reduce(
                LLacc[0:1, 0:1], LLp[0:1, 0:ZCOLS],
                axis=mybir.AxisListType.X, op=mybir.AluOpType.add,
            ).then_inc(dve_sem, 1)

        @block.gpsimd
        def _(gp):
            for t in range(1, Tm + 1):
                bn = (t + 1) % 2
                if t >= 2:
                    gp.wait_ge(sloc_sem, 16 * NC * (t - 1))
                    for d in range(NC):
                        gp.wait_ge(rv[(t - 1) % 2][d], 2 * (t // 2))
                    gp.sem_inc(ready_sem, 1)
                if t == Tm:
                    break
                for d in range(NC):
                    rdests = [None] * NC
                    rdests[d] = (0, d)
                    gp.remote_dma_broadcast(
                        out_ap=R[bn][:, 32 * d:32 * (d + 1)],
                        in_ap=Sb[t % 2][:, :],
                        remote_sem=rv[t % 2][d], local_sem=sloc_sem, rdests=rdests,
                    ).then_inc(prep_sem, 1)
                gp.wait_ge(prep_sem, NC * t)
                gp.wait_ge(dve_sem, t)
                gp.trigger_dma(count=NC)
            # bulk epilogue: reshape zlog [1, TmPad] -> zlog2 [128, ZCOLS]
            gp.wait_ge(dve_sem, Tm)
            gp.dma_start(zlog2[:, :], zlog[0:1, :]).then_inc(fin_sem, 16)
            gp.wait_ge(dve_sem, Tm + 1)
            gp.dma_start(out_d[:, :], LLacc[0:1, 0:1]).then_inc(out_sem, 16)
            gp.wait_ge(out_sem, 16)

    nc.finalize()
    return nc


def _prep_inputs(probt, transition, pi, sigma, nsteps):
    """Per-core rearranged input arrays (all host-side numpy)."""
    jl = _jlocal()
    p = np.arange(128)
    E0 = np.exp(probt[0].astype(np.float64))
    s0 = (np.clip(pi.astype(np.float64), EPS, None) * E0).astype(np.float32)

    in_maps = []
    for k in range(NC):
        rows = np.zeros(S, dtype=np.int64)
        for ch in range(CH):
            snd = sigma[k][ch // 2]
            rows[ch * 128 + p] = SL * snd + jl[:, ch % 2]
        # A slice as stationary blocks: Abuf[p, (c*2+h)*128 + m] =
        # A[row(c, p), SL*k + 128h + m]
        Ak = transition[rows][:, SL * k:SL * (k + 1)].astype(np.float32)
        Abuf = (
            Ak.reshape(CH, 128, 2, 128)
            .transpose(1, 0, 2, 3)
            .reshape(128, CH * 2 * 128)
        ).astype(float8_e4m3)
        cols = SL * k + jl
        Ek = probt[:nsteps][:, cols]               # [Tm, 128, 2]
        Ebuf = np.ascontiguousarray(
            Ek.transpose(1, 0, 2).reshape(128, F * nsteps)
        ).astype(np.float32)
        r0 = np.zeros((128, 16 * CH), dtype=float8_e4m3)
        r0[:, 0:16 * CH:16] = s0[rows].reshape(CH, 128).T.astype(float8_e4m3)
        in_maps.append({"a": np.ascontiguousarray(Abuf), "e": Ebuf, "r0": r0})
    return in_maps, s0


def _run(probt, transition, pi, nsteps):
    sigma = _probe_sigma()
    nc = _build_main(nsteps)
    in_maps, _ = _prep_inputs(probt, transition, pi, sigma, nsteps)
    res = run_bass_kernel_spmd(nc, in_maps, core_ids=list(range(NC)))
    return np.float32(res.results[0]["out"][0, 0])


def kernel(probt, transition, pi):
    ll = _run(np.asarray(probt), np.asarray(transition), np.asarray(pi), T)
    return np.float32(ll)



# revision 5
# speedup vs baseline: 2.6597x; 2.6597x over previous
"""HMM forward-algorithm (nn_ForwardBackward) Trainium2 Bass kernel.

The reference is a strictly sequential scan of T=8192 steps, each a matvec of
the state row-vector with the SxS transition matrix followed by a rescale.
Rewritten in linear space with HOST-SIDE predictive normalization:

    s_t = (s_{t-1} @ A) * Etil_t,   Etil_t = exp(probt[t]) / Khat_t
    Khat_t = (exp(probt[t]) . colsum(A)) / S     (host-precomputed)
    LL  = log(sum s_{T-1}) + log(Z0) + sum_t log(Khat_t)

Because Khat_t predicts the per-step growth of sum(s) to within a few percent,
the bf16 state never drifts out of range and NO on-device normalization is
needed: no Z matvec, no reciprocal, no per-step log.  The device does only

    PE:   s_chunk matvecs into PSUM            (32 bf16 matmuls, ~0 cost)
    DVE:  Sb = P * Etil_t   (2 single-column tensor_tensor, PSUM->SBUF bf16)
    Pool: trigger pre-prepared SWDGE remote broadcast (XOR-slot, 8 descs)

so the steady-state critical path is 3 semaphore hops per step:
PE -> DVE -> Pool-trigger -> remote arrival -> PE.

Tensor-parallel over 8 NeuronCores: core k owns a 256-column slice of A
(SBUF-resident, bf16) in weights-stationary [128,128] blocks; state chunks
[128,1] move; outputs accumulate partition-major in PSUM.  Each core
broadcasts its bf16 [128,2] slice to all 8 cores with SWDGE remote DMA
(XOR-slot addressing, per-slot parity-split arrival semaphores, runtime probe
for the logical->physical core permutation).  The single final sum(s) comes
from one ones-matmul round; log and constant-folding happen on the host.
"""

import functools
import numpy as np
from ml_dtypes import bfloat16

from concourse import bass, bacc, mybir
from concourse.bass_utils import run_bass_kernel_spmd

S = 2048
T = 8192
NC = 8
SL = S // NC          # 256 state columns per core
CH = 16               # contraction chunks of 128
EPS = 1e-8

f32 = mybir.dt.float32
bf16 = mybir.dt.bfloat16


def _build_probe():
    """Tiny SPMD kernel: every core broadcasts its id-tile to all 8 XOR slots."""
    nc = bacc.Bacc(None, target_bir_lowering=False, num_devices=NC)
    x = nc.declare_dram_parameter("x", [128, 1], f32, isOutput=False)
    y = nc.declare_dram_parameter("y", [128, NC], f32, isOutput=True)
    with (
        nc.sbuf_tensor("S", [128, 1], f32) as Sb,
        nc.sbuf_tensor("R", [128, NC], f32) as Rb,
        nc.semaphore("dma_sem") as dma_sem,
        nc.semaphore("recv_sem") as recv_sem,
        nc.semaphore("send_local") as send_local,
        nc.semaphore("prep_sem") as prep_sem,
        nc.semaphore("out_sem") as out_sem,
        nc.Block() as block,
    ):
        @block.sync
        def _(sync):
            sync.dma_start(Sb[:, :], x[:, :]).then_inc(dma_sem, 16)

        @block.gpsimd
        def _(gp):
            for d in range(NC):
                rdests = [None] * NC
                rdests[d] = (0, d)
                gp.remote_dma_broadcast(
                    out_ap=Rb[:, d:d + 1], in_ap=Sb[:, :],
                    remote_sem=recv_sem, local_sem=send_local, rdests=rdests,
                ).then_inc(prep_sem, 1)
            gp.wait_ge(prep_sem, NC)
            gp.wait_ge(dma_sem, 16)
            gp.trigger_dma(count=NC)
            gp.wait_ge(recv_sem, 16)
            gp.dma_start(y[:, :], Rb[:, :]).then_inc(out_sem, 16)
            gp.wait_ge(out_sem, 16)
            gp.wait_ge(send_local, 16 * NC)
    nc.finalize()
    return nc


@functools.lru_cache(maxsize=1)
def _probe_sigma():
    """sigma[j][d] = logical id of the core whose slice lands in slot d of core j."""
    nc = _build_probe()
    ins = [{"x": np.full((128, 1), k, np.float32)} for k in range(NC)]
    res = run_bass_kernel_spmd(nc, ins, core_ids=list(range(NC)))
    sigma = []
    for j in range(NC):
        out = res.results[j]["y"]
        sigma.append(tuple(int(out[0, d]) for d in range(NC)))
    return tuple(sigma)


@functools.lru_cache(maxsize=8)
def _build_main(nsteps):
    """HMM forward kernel, unrolled.  Time indices t = 0..nsteps-1; rounds
    t = 1..nsteps-1 are compute+exchange; a final ones-matvec round sums the
    last state; the scalar sum is DMAd out (log + constants on host)."""
    Tm = nsteps
    nc = bacc.Bacc(None, target_bir_lowering=False, num_devices=NC,
                   dynamic_dma_scratch_size=65536)

    a_d = nc.declare_dram_parameter("a", [128, CH * 2 * 128], bf16, isOutput=False)
    e_d = nc.declare_dram_parameter("e", [128, 2 * Tm], f32, isOutput=False)
    r0_d = nc.declare_dram_parameter("r0", [128, CH], bf16, isOutput=False)
    out_d = nc.declare_dram_parameter("out", [1, 1], f32, isOutput=True)

    from contextlib import ExitStack
    with ExitStack() as es:
        ec = es.enter_context
        Abuf = ec(nc.sbuf_tensor("Abuf", [128, CH * 2 * 128], bf16))
        Ebuf = ec(nc.sbuf_tensor("Ebuf", [128, 2 * Tm], f32))
        R = [ec(nc.sbuf_tensor(f"R{i}", [128, CH], bf16)) for i in range(2)]
        Sb = [ec(nc.sbuf_tensor(f"S{i}", [128, 2], bf16)) for i in range(2)]
        ones = ec(nc.sbuf_tensor("ones", [128, 128], bf16))
        outb = ec(nc.sbuf_tensor("outb", [1, 1], f32))
        P = [[ec(nc.psum_tensor(f"P{i}_{h}", [128, 512], f32)) for h in range(2)] for i in range(2)]
        Zp = ec(nc.psum_tensor("Zp", [128, 512], f32))
        sem = lambda n: ec(nc.semaphore(n))
        ld_sem = sem("ld_sem"); misc_sem = sem("misc_sem")
        mm_sem = sem("mm_sem"); dve_sem = sem("dve_sem")
        sloc_sem = sem("sloc_sem"); prep_sem = sem("prep_sem")
        zfin_sem = sem("zfin_sem"); fin_sem = sem("fin_sem")
        out_sem = sem("out_sem")
        # remote-arrival sems, split by parity of the R buffer written:
        # round tau's broadcast writes R[(tau+1)%2] and bumps rv[(tau+1)%2][d]
        # by 2 on every core (slot d).
        rv = [[sem(f"rv{p}_{d}") for d in range(NC)] for p in range(2)]
        block = ec(nc.Block())

        @block.sync
        def _(sync):
            sync.dma_start(Abuf[:, :], a_d[:, :]).then_inc(ld_sem, 16)
            sync.dma_start(Ebuf[:, :], e_d[:, :]).then_inc(ld_sem, 16)
            sync.dma_start(R[1][:, :], r0_d[:, :]).then_inc(ld_sem, 16)

        @block.tensor
        def _(te):
            te.wait_ge(ld_sem, 48)
            te.wait_ge(misc_sem, 1)
            for t in range(1, Tm + 1):
                bt = t % 2
                if t >= 2:
                    narr = t // 2
                    for d in range(NC):
                        te.wait_ge(rv[bt][d], 2 * narr)
                if t == Tm:
                    break
                if t >= 3:
                    te.wait_ge(dve_sem, 2 * (t - 2))
                last = None
                for c in range(CH):
                    for h in range(2):
                        last = nc.tensor.matmul(
                            P[bt][h][:, 0:1],
                            Abuf[:, (c * 2 + h) * 128:(c * 2 + h + 1) * 128],
                            R[bt][:, c:c + 1],
                            start=(c == 0), stop=(c == CH - 1),
                        )
                last.then_inc(mm_sem, 1)
            # final round: Z = sum(s_{Tm-1}) via ones-matvec (all partitions)
            bt = Tm % 2
            lastz = None
            for c in range(CH):
                lastz = nc.tensor.matmul(
                    Zp[:, 0:1], ones[:, 0:128], R[bt][:, c:c + 1],
                    start=(c == 0), stop=(c == CH - 1),
                )
            lastz.then_inc(zfin_sem, 1)

        @block.vector
        def _(ve):
            ve.memset(ones[:, :], 1.0).then_inc(misc_sem, 1)
            ve.wait_ge(ld_sem, 48)
            for t in range(1, Tm):
                bt = t % 2
                ve.wait_ge(mm_sem, t)
                if t >= 3:
                    ve.wait_ge(sloc_sem, 128 * (t - 2))
                ve.tensor_tensor(
                    Sb[bt][:, 0:1], P[bt][0][:, 0:1], Ebuf[:, 2 * t:2 * t + 1],
                    op=mybir.AluOpType.mult,
                ).then_inc(dve_sem, 1)
                ve.tensor_tensor(
                    Sb[bt][:, 1:2], P[bt][1][:, 0:1], Ebuf[:, 2 * t + 1:2 * t + 2],
                    op=mybir.AluOpType.mult,
                ).then_inc(dve_sem, 1)
            ve.wait_ge(zfin_sem, 1)
            ve.tensor_copy(outb[0:1, 0:1], Zp[0:1, 0:1]).then_inc(fin_sem, 1)

        @block.gpsimd
        def _(gp):
            def prep_round(rnd):
                # round rnd broadcasts Sb[rnd%2] into R[(rnd+1)%2] slot cols
                bn = (rnd + 1) % 2
                for d in range(NC):
                    rdests = [None] * NC
                    rdests[d] = (0, d)
                    gp.remote_dma_broadcast(
                        out_ap=R[bn][:, 2 * d:2 * (d + 1)],
                        in_ap=Sb[rnd % 2][:, :],
                        remote_sem=rv[bn][d], local_sem=sloc_sem, rdests=rdests,
                    ).then_inc(prep_sem, 1)

            prep_round(1)
            if Tm >= 3:
                prep_round(2)
            for t in range(1, Tm):
                gp.wait_ge(prep_sem, 8 * min(t + 1, Tm - 1))
                if t >= 2:
                    gp.wait_ge(sloc_sem, 16 * NC * (t - 1))
                gp.wait_ge(dve_sem, 2 * t)
                gp.trigger_dma(count=NC)
                if t + 2 <= Tm - 1:
                    prep_round(t + 2)
            gp.wait_ge(fin_sem, 1)
            gp.dma_start(out_d[:, :], outb[0:1, 0:1]).then_inc(out_sem, 16)
            gp.wait_ge(out_sem, 16)
            gp.wait_ge(sloc_sem, 16 * NC * (Tm - 1))

    nc.finalize()
    return nc


def _host_prep(probt, transition, pi, nsteps):
    """Host-side scaling: returns (E-tilde [Tm,S] f32, s0 bf16 [S], llconst)."""
    Tm = nsteps
    E = np.exp(probt[:Tm].astype(np.float64))
    colsum = transition.astype(np.float64).sum(axis=0)
    Khat = (E @ colsum) / S                                   # [Tm]
    Etil = (E / Khat[:, None]).astype(np.float32)
    s0 = np.clip(pi.astype(np.float64), EPS, None) * E[0]
    Z0h = s0.sum()
    s0n = (s0 / Z0h).astype(np.float32).astype(bfloat16)
    llconst = float(np.log(Z0h) + np.log(Khat[1:Tm]).sum())
    return Etil, s0n, llconst


def _prep_inputs(probt, transition, pi, sigma, nsteps):
    """Per-core rearranged input arrays (all host-side numpy)."""
    Tm = nsteps
    p = np.arange(128)
    Etil, s0n, llconst = _host_prep(probt, transition, pi, Tm)
    Abf = transition.astype(bfloat16)

    in_maps = []
    for k in range(NC):
        # contraction chunk j (R column j) holds sender sigma[k][j//2]'s
        # half j%2: global state rows 256*sigma[k][j//2] + 128*(j%2) + q
        rows = np.zeros(S, dtype=np.int64)
        for j in range(CH):
            snd = sigma[k][j // 2]
            rows[j * 128 + p] = SL * snd + 128 * (j % 2) + p
        Ak = Abf[rows][:, SL * k:SL * (k + 1)].astype(np.float32)
        Abuf = (
            Ak.reshape(CH, 128, 2, 128)
            .transpose(1, 0, 2, 3)
            .reshape(128, CH * 2 * 128)
        ).astype(bfloat16)
        cols = (SL * k + 128 * np.arange(2)[None, :] + p[:, None])  # [128, 2]
        Ek = Etil[:, cols]                                   # [Tm, 128, 2]
        Ebuf = np.ascontiguousarray(
            Ek.transpose(1, 0, 2).reshape(128, 2 * Tm)
        ).astype(np.float32)
        r0 = np.ascontiguousarray(
            s0n[rows].reshape(CH, 128).T
        ).astype(bfloat16)
        in_maps.append({"a": np.ascontiguousarray(Abuf), "e": Ebuf, "r0": r0})
    return in_maps, llconst


def _run(probt, transition, pi, nsteps):
    sigma = _probe_sigma()
    nc = _build_main(nsteps)
    in_maps, llconst = _prep_inputs(probt, transition, pi, sigma, nsteps)
    res = run_bass_kernel_spmd(nc, in_maps, core_ids=list(range(NC)))
    zfin = float(res.results[0]["out"][0, 0])
    return np.float32(np.log(zfin) + llconst)


def kernel(probt, transition, pi):
    ll = _run(np.asarray(probt), np.asarray(transition), np.asarray(pi), T)
    return np.float32(ll)


# revision 7
# speedup vs baseline: 2.8048x; 1.0546x over previous
"""HMM forward-algorithm (nn_ForwardBackward) Trainium2 Bass kernel.

The reference is a strictly sequential scan of T=8192 steps, each a matvec of
the state row-vector with the SxS transition matrix followed by a rescale.
Rewritten in linear space with HOST-SIDE predictive normalization:

    s_t = (s_{t-1} @ A) * Etil_t,   Etil_t = exp(probt[t]) / Khat_t
    Khat_t = (exp(probt[t]) . colsum(A)) / S     (host-precomputed)
    LL  = log(sum s_{T-1}) + log(Z0) + sum_t log(Khat_t)

Because Khat_t predicts the per-step growth of sum(s) to within a few percent,
the bf16 state never drifts out of range and NO on-device normalization is
needed: no Z matvec, no reciprocal, no per-step log.  The device does only

    PE:   s_chunk matvecs into PSUM            (32 bf16 matmuls, ~0 cost)
    DVE:  Sb = P * Etil_t   (2 single-column tensor_tensor, PSUM->SBUF bf16)
    Pool: trigger pre-prepared SWDGE remote broadcast (XOR-slot, 8 descs)

so the steady-state critical path is 3 semaphore hops per step:
PE -> DVE -> Pool-trigger -> remote arrival -> PE.

Tensor-parallel over 8 NeuronCores: core k owns a 256-column slice of A
(SBUF-resident, bf16) in weights-stationary [128,128] blocks; state chunks
[128,1] move; outputs accumulate partition-major in PSUM.  Each core
broadcasts its bf16 [128,2] slice to all 8 cores with SWDGE remote DMA
(XOR-slot addressing, per-slot parity-split arrival semaphores, runtime probe
for the logical->physical core permutation).  The single final sum(s) comes
from one ones-matmul round; log and constant-folding happen on the host.
"""

import functools
import numpy as np
from ml_dtypes import bfloat16

from concourse import bass, bacc, mybir
from concourse.bass_utils import run_bass_kernel_spmd

S = 2048
T = 8192
NC = 8
SL = S // NC          # 256 state columns per core
CH = 16               # contraction chunks of 128
EPS = 1e-8

f32 = mybir.dt.float32
bf16 = mybir.dt.bfloat16


def _build_probe():
    """Tiny SPMD kernel: every core broadcasts its id-tile to all 8 XOR slots."""
    nc = bacc.Bacc(None, target_bir_lowering=False, num_devices=NC)
    x = nc.declare_dram_parameter("x", [128, 1], f32, isOutput=False)
    y = nc.declare_dram_parameter("y", [128, NC], f32, isOutput=True)
    with (
        nc.sbuf_tensor("S", [128, 1], f32) as Sb,
        nc.sbuf_tensor("R", [128, NC], f32) as Rb,
        nc.semaphore("dma_sem") as dma_sem,
        nc.semaphore("recv_sem") as recv_sem,
        nc.semaphore("send_local") as send_local,
        nc.semaphore("prep_sem") as prep_sem,
        nc.semaphore("out_sem") as out_sem,
        nc.Block() as block,
    ):
        @block.sync
        def _(sync):
            sync.dma_start(Sb[:, :], x[:, :]).then_inc(dma_sem, 16)

        @block.gpsimd
        def _(gp):
            for d in range(NC):
                rdests = [None] * NC
                rdests[d] = (0, d)
                gp.remote_dma_broadcast(
                    out_ap=Rb[:, d:d + 1], in_ap=Sb[:, :],
                    remote_sem=recv_sem, local_sem=send_local, rdests=rdests,
                ).then_inc(prep_sem, 1)
            gp.wait_ge(prep_sem, NC)
            gp.wait_ge(dma_sem, 16)
            gp.trigger_dma(count=NC)
            gp.wait_ge(recv_sem, 16)
            gp.dma_start(y[:, :], Rb[:, :]).then_inc(out_sem, 16)
            gp.wait_ge(out_sem, 16)
            gp.wait_ge(send_local, 16 * NC)
    nc.finalize()
    return nc


@functools.lru_cache(maxsize=1)
def _probe_sigma():
    """sigma[j][d] = logical id of the core whose slice lands in slot d of core j."""
    nc = _build_probe()
    ins = [{"x": np.full((128, 1), k, np.float32)} for k in range(NC)]
    res = run_bass_kernel_spmd(nc, ins, core_ids=list(range(NC)))
    sigma = []
    for j in range(NC):
        out = res.results[j]["y"]
        sigma.append(tuple(int(out[0, d]) for d in range(NC)))
    return tuple(sigma)


@functools.lru_cache(maxsize=8)
def _build_main(nsteps):
    """HMM forward kernel, unrolled.  Time indices t = 0..nsteps-1; rounds
    t = 1..nsteps-1 are compute+exchange; a final ones-matvec round sums the
    last state; the scalar sum is DMAd out (log + constants on host)."""
    Tm = nsteps
    nc = bacc.Bacc(None, target_bir_lowering=False, num_devices=NC,
                   dynamic_dma_scratch_size=65536)

    a_d = nc.declare_dram_parameter("a", [128, CH * 2 * 128], bf16, isOutput=False)
    e_d = nc.declare_dram_parameter("e", [128, 2 * Tm], f32, isOutput=False)
    r0_d = nc.declare_dram_parameter("r0", [128, CH], bf16, isOutput=False)
    out_d = nc.declare_dram_parameter("out", [1, 1], f32, isOutput=True)

    from contextlib import ExitStack
    with ExitStack() as es:
        ec = es.enter_context
        Abuf = ec(nc.sbuf_tensor("Abuf", [128, CH * 2 * 128], bf16))
        Ebuf = ec(nc.sbuf_tensor("Ebuf", [128, 2 * Tm], f32))
        R = [ec(nc.sbuf_tensor(f"R{i}", [128, CH], bf16)) for i in range(2)]
        Sb = [ec(nc.sbuf_tensor(f"S{i}", [128, 2], bf16)) for i in range(2)]
        ones = ec(nc.sbuf_tensor("ones", [128, 128], bf16))
        outb = ec(nc.sbuf_tensor("outb", [1, 1], f32))
        P = [[ec(nc.psum_tensor(f"P{i}_{h}", [128, 512], f32)) for h in range(2)] for i in range(2)]
        Zp = ec(nc.psum_tensor("Zp", [128, 512], f32))
        sem = lambda n: ec(nc.semaphore(n))
        ld_sem = sem("ld_sem"); misc_sem = sem("misc_sem")
        mm_sem = sem("mm_sem"); dve_sem = sem("dve_sem")
        sloc_sem = sem("sloc_sem"); prep_sem = sem("prep_sem")
        zfin_sem = sem("zfin_sem"); fin_sem = sem("fin_sem")
        ldb_sem = sem("ldb_sem")
        out_sem = sem("out_sem")
        # remote-arrival sems, split by parity of the R buffer written:
        # round tau's broadcast writes R[(tau+1)%2] and bumps rv[(tau+1)%2][d]
        # by 2 on every core (slot d).
        rv = [[sem(f"rv{p}_{d}") for d in range(NC)] for p in range(2)]
        block = ec(nc.Block())

        # Only the first ECUT steps' emissions gate loop start; the bulk
        # streams in behind (done by ~step 120 << ECUT at 200ns/step).
        ECUT = min(Tm, 512)

        @block.sync
        def _(sync):
            sync.dma_start(Abuf[:, :], a_d[:, :]).then_inc(ld_sem, 16)
            sync.dma_start(Ebuf[:, 0:2 * ECUT], e_d[:, 0:2 * ECUT]).then_inc(ld_sem, 16)
            sync.dma_start(R[1][:, :], r0_d[:, :]).then_inc(ld_sem, 16)
            if ECUT < Tm:
                sync.dma_start(
                    Ebuf[:, 2 * ECUT:], e_d[:, 2 * ECUT:]
                ).then_inc(ldb_sem, 16)

        @block.tensor
        def _(te):
            te.wait_ge(ld_sem, 48)
            te.wait_ge(misc_sem, 1)
            for t in range(1, Tm + 1):
                bt = t % 2
                if t >= 2:
                    narr = t // 2
                    for d in range(NC):
                        te.wait_ge(rv[bt][d], 2 * narr)
                if t == Tm:
                    break
                if t >= 3:
                    te.wait_ge(dve_sem, 2 * (t - 2))
                last = None
                for c in range(CH):
                    for h in range(2):
                        last = nc.tensor.matmul(
                            P[bt][h][:, 0:1],
                            Abuf[:, (c * 2 + h) * 128:(c * 2 + h + 1) * 128],
                            R[bt][:, c:c + 1],
                            start=(c == 0), stop=(c == CH - 1),
                        )
                last.then_inc(mm_sem, 1)
            # final round: Z = sum(s_{Tm-1}) via ones-matvec (all partitions)
            bt = Tm % 2
            lastz = None
            for c in range(CH):
                lastz = nc.tensor.matmul(
                    Zp[:, 0:1], ones[:, 0:128], R[bt][:, c:c + 1],
                    start=(c == 0), stop=(c == CH - 1),
                )
            lastz.then_inc(zfin_sem, 1)

        @block.vector
        def _(ve):
            ve.memset(ones[:, :], 1.0).then_inc(misc_sem, 1)
            ve.wait_ge(ld_sem, 48)
            ECUT = min(Tm, 512)
            for t in range(1, Tm):
                bt = t % 2
                if t == ECUT and ECUT < Tm:
                    ve.wait_ge(ldb_sem, 16)
                ve.wait_ge(mm_sem, t)
                if t >= 3:
                    ve.wait_ge(sloc_sem, 128 * (t - 2))
                ve.tensor_tensor(
                    Sb[bt][:, 0:1], P[bt][0][:, 0:1], Ebuf[:, 2 * t:2 * t + 1],
                    op=mybir.AluOpType.mult,
                ).then_inc(dve_sem, 1)
                ve.tensor_tensor(
                    Sb[bt][:, 1:2], P[bt][1][:, 0:1], Ebuf[:, 2 * t + 1:2 * t + 2],
                    op=mybir.AluOpType.mult,
                ).then_inc(dve_sem, 1)
            ve.wait_ge(zfin_sem, 1)
            ve.tensor_copy(outb[0:1, 0:1], Zp[0:1, 0:1]).then_inc(fin_sem, 1)

        @block.gpsimd
        def _(gp):
            def prep_round(rnd):
                # round rnd broadcasts Sb[rnd%2] into R[(rnd+1)%2] slot cols
                bn = (rnd + 1) % 2
                for d in range(NC):
                    rdests = [None] * NC
                    rdests[d] = (0, d)
                    gp.remote_dma_broadcast(
                        out_ap=R[bn][:, 2 * d:2 * (d + 1)],
                        in_ap=Sb[rnd % 2][:, :],
                        remote_sem=rv[bn][d], local_sem=sloc_sem, rdests=rdests,
                    ).then_inc(prep_sem, 1)

            prep_round(1)
            if Tm >= 3:
                prep_round(2)
            for t in range(1, Tm):
                gp.wait_ge(prep_sem, 8 * min(t + 1, Tm - 1))
                if t >= 2:
                    gp.wait_ge(sloc_sem, 16 * NC * (t - 1))
                gp.wait_ge(dve_sem, 2 * t)
                gp.trigger_dma(count=NC)
                if t + 2 <= Tm - 1:
                    prep_round(t + 2)
            gp.wait_ge(fin_sem, 1)
            gp.dma_start(out_d[:, :], outb[0:1, 0:1]).then_inc(out_sem, 16)
            gp.wait_ge(out_sem, 16)
            gp.wait_ge(sloc_sem, 16 * NC * (Tm - 1))

    nc.finalize()
    return nc


def _host_prep(probt, transition, pi, nsteps):
    """Host-side scaling: returns (E-tilde [Tm,S] f32, s0 bf16 [S], llconst)."""
    Tm = nsteps
    E = np.exp(probt[:Tm].astype(np.float64))
    colsum = transition.astype(np.float64).sum(axis=0)
    Khat = (E @ colsum) / S                                   # [Tm]
    Etil = (E / Khat[:, None]).astype(np.float32)
    s0 = np.clip(pi.astype(np.float64), EPS, None) * E[0]
    Z0h = s0.sum()
    s0n = (s0 / Z0h).astype(np.float32).astype(bfloat16)
    llconst = float(np.log(Z0h) + np.log(Khat[1:Tm]).sum())
    return Etil, s0n, llconst


def _prep_inputs(probt, transition, pi, sigma, nsteps):
    """Per-core rearranged input arrays (all host-side numpy)."""
    Tm = nsteps
    p = np.arange(128)
    Etil, s0n, llconst = _host_prep(probt, transition, pi, Tm)
    Abf = transition.astype(bfloat16)

    in_maps = []
    for k in range(NC):
        # contraction chunk j (R column j) holds sender sigma[k][j//2]'s
        # half j%2: global state rows 256*sigma[k][j//2] + 128*(j%2) + q
        rows = np.zeros(S, dtype=np.int64)
        for j in range(CH):
            snd = sigma[k][j // 2]
            rows[j * 128 + p] = SL * snd + 128 * (j % 2) + p
        Ak = Abf[rows][:, SL * k:SL * (k + 1)].astype(np.float32)
        Abuf = (
            Ak.reshape(CH, 128, 2, 128)
            .transpose(1, 0, 2, 3)
            .reshape(128, CH * 2 * 128)
        ).astype(bfloat16)
        cols = (SL * k + 128 * np.arange(2)[None, :] + p[:, None])  # [128, 2]
        Ek = Etil[:, cols]                                   # [Tm, 128, 2]
        Ebuf = np.ascontiguousarray(
            Ek.transpose(1, 0, 2).reshape(128, 2 * Tm)
        ).astype(np.float32)
        r0 = np.ascontiguousarray(
            s0n[rows].reshape(CH, 128).T
        ).astype(bfloat16)
        in_maps.append({"a": np.ascontiguousarray(Abuf), "e": Ebuf, "r0": r0})
    return in_maps, llconst


def _run(probt, transition, pi, nsteps):
    sigma = _probe_sigma()
    nc = _build_main(nsteps)
    in_maps, llconst = _prep_inputs(probt, transition, pi, sigma, nsteps)
    res = run_bass_kernel_spmd(nc, in_maps, core_ids=list(range(NC)))
    zfin = float(res.results[0]["out"][0, 0])
    return np.float32(np.log(zfin) + llconst)


def kernel(probt, transition, pi):
    ll = _run(np.asarray(probt), np.asarray(transition), np.asarray(pi), T)
    return np.float32(ll)
